# revision 6
# baseline (speedup 1.0000x reference)
"""Trainium2 Bass kernel for nn_CConv (causal depthwise FFT-conv, 512 taps).

The reference's FFT conv is exactly a causal depthwise conv1d with a
512-step learned init state prepended:
    out[b,t,c] = sum_k f[k,c] * xc[b, t+512-k, c],  xc = concat(init, x)

Mapping to the tensor engine (per channel c):
    out[i, (m0,b)] = sum_q  W_q[j,i].T @ X[j, (m0+q, b)]
where W_q[j,i] = f[i-j+128*(4-q), c] are 5 Toeplitz slices of a
[128 x 640] band built on the host, and X is the natural time-tiled
x (partition = t%128, columns = (time chunk, batch)).  The 5 matmuls
accumulate in PSUM.  Channels are sharded 8 ways across cores; inside a
core, channels stream in groups of 8 (fused x+band slab DMA -> 40
matmuls -> PSUM->SBUF copy -> output DMA).

Numerics: fp16 operands with fp32 PSUM accumulation.  The filter is
prescaled by 32 and x by 1/32 (exact powers of two; out = sum f*x
unchanged) so both operands sit far from the fp16 subnormal range.
"""

import os

import numpy as np

import concourse.bacc as bacc
import concourse.mybir as mybir
from concourse.bass_utils import run_bass_kernel_spmd
from concourse.tile import TileContext

B, L, D, CLEN = 4, 4096, 1024, 512
NCORES = 8
DSH = D // NCORES            # 128 channels per core
GCH = 8                      # channels per group
NG = DSH // GCH              # 16 groups per core
MIN = (CLEN + L) // 128      # 36 input chunks (4 init + 32 x)
MOUT = L // 128              # 32 output blocks
NQ = 5                       # contraction chunks per output block
XW = MIN * B                 # 144 x columns per channel
BW = 640                     # band columns per channel
CW = XW + BW                 # 784 slab columns per channel
SCALE = 32.0

_CACHE = {}
LAST_RESULTS = None          # BassKernelResults of the most recent run


def _build_bass():
    # Bacc (not plain Bass): its compile() legalizes sync waits (>1 wait per
    # instruction gets split into InstEventSemaphore), which walrus requires.
    nc = bacc.Bacc(None, target_bir_lowering=False)
    f16, f32 = mybir.dt.float16, mybir.dt.float32
    sd = nc.declare_dram_parameter("slab", [NG, 128, GCH, CW], f16, isOutput=False)
    od = nc.declare_dram_parameter("out", [NG, 128, GCH, 128], f32, isOutput=True)

    with TileContext(nc) as tc:
        with (
            tc.tile_pool(name="sp", bufs=4) as sp,
            tc.tile_pool(name="op", bufs=4) as op,
            tc.tile_pool(name="pp", bufs=4, space="PSUM") as pp,
        ):
            for g in range(NG):
                st = sp.tile([128, GCH, CW], f16)
                nc.sync.dma_start(out=st[:], in_=sd[g])
                ot = op.tile([128, GCH, 128], f32)
                for c in range(GCH):
                    ps = pp.tile([128, 128], f32)
                    for q in range(NQ):
                        nc.tensor.matmul(
                            ps[:],
                            lhsT=st[:, c, XW + 128 * (4 - q) : XW + 128 * (5 - q)],
                            rhs=st[:, c, 4 * q : 4 * q + 128],
                            start=(q == 0),
                            stop=(q == NQ - 1),
                        )
                    nc.vector.tensor_copy(out=ot[:, c, :], in_=ps[:])
                nc.sync.dma_start(out=od[g], in_=ot[:])
    nc.finalize()  # Bacc.compile(): reg alloc + sync-wait legalization
    return nc


def _prep_inputs(x, last_input_init, filt):
    """Host-side: cast/scale to fp16 and prearrange into the exact SBUF
    layout so every DMA is a contiguous line-rate copy.

    slab[core, g, j, c, 0:144]   = xc[b, 128*m + j, ch] / 32   at col m*4+b
    slab[core, g, j, c, 144:784] = 32*f[u - j, ch]             at col 144+u
    """
    x = np.asarray(x, dtype=np.float32)
    init = np.asarray(last_input_init, dtype=np.float32)
    filt = np.asarray(filt, dtype=np.float32)

    xc = np.concatenate(
        [np.broadcast_to(init[None], (B, CLEN, D)), x], axis=1
    )  # [B, 4608, D]
    xh = (xc * np.float32(1.0 / SCALE)).astype(np.float16)
    xr = xh.reshape(B, MIN, 128, D)                      # [b, m, j, ch]
    xt = xr.transpose(3, 2, 1, 0)                        # [ch, j, m, b]
    xt = xt.reshape(D, 128, XW)                          # [ch, j, m*4+b]

    fs = (filt * np.float32(SCALE)).astype(np.float16)   # [512, D]
    pf = np.zeros((D, 767), np.float16)
    pf[:, 127:639] = fs.T
    jj = np.arange(128)
    uu = np.arange(BW)
    idx = 127 - jj[:, None] + uu[None, :]                # [128, 640] in [0, 767)
    band = pf[:, idx]                                    # [ch, j, u]

    slab = np.empty((D, 128, CW), np.float16)
    slab[:, :, :XW] = xt
    slab[:, :, XW:] = band
    slab = slab.reshape(NCORES, NG, GCH, 128, CW).transpose(0, 1, 3, 2, 4)
    return np.ascontiguousarray(slab)


def kernel(x, last_input_init, filt):
    global LAST_RESULTS
    if "nc" not in _CACHE:
        _CACHE["nc"] = _build_bass()
    nc = _CACHE["nc"]

    slab = _prep_inputs(x, last_input_init, filt)
    in_maps = [{"slab": slab[core]} for core in range(NCORES)]

    trace = bool(os.environ.get("BASS_TRACE"))
    res = run_bass_kernel_spmd(nc, in_maps, list(range(NCORES)), trace=trace)
    LAST_RESULTS = res

    outs = []
    for core in range(NCORES):
        o = res.results[core]["out"]                     # [NG, 128, GCH, 128] f32
        o = o.reshape(NG, 128, GCH, MOUT, B)             # [g, i, c, m0, b]
        o = o.transpose(4, 3, 1, 0, 2)                   # [b, m0, i, g, c]
        outs.append(o.reshape(B, L, DSH))
    out = np.concatenate(outs, axis=2)
    return np.ascontiguousarray(out, dtype=np.float32)


# revision 9
# speedup vs baseline: 1.1069x; 1.1069x over previous
"""Trainium2 Bass kernel for nn_CConv (causal depthwise FFT-conv, 512 taps).

The reference's FFT conv is exactly a causal depthwise conv1d with a
512-step learned init state prepended:
    out[b,t,c] = sum_k f[k,c] * xc[b, t+512-k, c],  xc = concat(init, x)

Mapping to the tensor engine (per channel c):
    out[i, (m0,b)] = sum_q  W_q[j,i].T @ X[j, (m0+q, b)]
where W_q[j,i] = f[i-j+128*(4-q), c] are 5 Toeplitz slices of a
[128 x 640] band built on the host, and X is the natural time-tiled
x (partition = t%128, columns = (time chunk, batch)).  The 5 matmuls
accumulate in PSUM.  Channels are sharded 8 ways across cores; inside a
core, channels stream in groups of 8 (fused x+band slab DMA -> 40
matmuls -> PSUM->SBUF copy -> output DMA).

Numerics: fp16 operands with fp32 PSUM accumulation.  The filter is
prescaled by 32 and x by 1/32 (exact powers of two; out = sum f*x
unchanged) so both operands sit far from the fp16 subnormal range.
"""

import os

import numpy as np

import concourse.bacc as bacc
import concourse.mybir as mybir
from concourse.bass_utils import run_bass_kernel_spmd
from concourse.tile import TileContext

B, L, D, CLEN = 4, 4096, 1024, 512
NCORES = 8
DSH = D // NCORES            # 128 channels per core
GCH = 8                      # channels per group
NG = DSH // GCH              # 16 groups per core
MIN = (CLEN + L) // 128      # 36 input chunks (4 init + 32 x)
MOUT = L // 128              # 32 output blocks
NQ = 5                       # contraction chunks per output block
XW = MIN * B                 # 144 x columns per channel
BW = 640                     # band columns per channel
CW = XW + BW                 # 784 slab columns per channel
SCALE = 32.0

_CACHE = {}
LAST_RESULTS = None          # BassKernelResults of the most recent run


def _build_bass():
    # Bacc (not plain Bass): its compile() legalizes sync waits (>1 wait per
    # instruction gets split into InstEventSemaphore), which walrus requires.
    nc = bacc.Bacc(None, target_bir_lowering=False)
    f16, f32 = mybir.dt.float16, mybir.dt.float32
    sd = nc.declare_dram_parameter("slab", [NG, 128, GCH, CW], f16, isOutput=False)
    od = nc.declare_dram_parameter("out", [NG, 128, GCH, 128], f16, isOutput=True)

    with TileContext(nc) as tc:
        with (
            tc.tile_pool(name="sp", bufs=4) as sp,
            tc.tile_pool(name="op", bufs=4) as op,
            tc.tile_pool(name="pp", bufs=4, space="PSUM") as pp,
        ):
            for g in range(NG):
                st = sp.tile([128, GCH, CW], f16)
                if g == 0:
                    # split so the first channels' matmuls start sooner
                    nc.sync.dma_start(out=st[:, :2], in_=sd[g, :, :2])
                    nc.sync.dma_start(out=st[:, 2:], in_=sd[g, :, 2:])
                else:
                    nc.sync.dma_start(out=st[:], in_=sd[g])
                ot = op.tile([128, GCH, 128], f16)
                for c in range(GCH):
                    ps = pp.tile([128, 128], f32)
                    for q in range(NQ):
                        nc.tensor.matmul(
                            ps[:],
                            lhsT=st[:, c, XW + 128 * (4 - q) : XW + 128 * (5 - q)],
                            rhs=st[:, c, 4 * q : 4 * q + 128],
                            start=(q == 0),
                            stop=(q == NQ - 1),
                        )
                    nc.vector.tensor_copy(out=ot[:, c, :], in_=ps[:])
                nc.sync.dma_start(out=od[g], in_=ot[:])
    nc.finalize()  # Bacc.compile(): reg alloc + sync-wait legalization
    return nc


def _prep_inputs(x, last_input_init, filt):
    """Host-side: cast/scale to fp16 and prearrange into the exact SBUF
    layout so every DMA is a contiguous line-rate copy.

    slab[core, g, j, c, 0:144]   = xc[b, 128*m + j, ch] / 32   at col m*4+b
    slab[core, g, j, c, 144:784] = 32*f[u - j, ch]             at col 144+u
    """
    x = np.asarray(x, dtype=np.float32)
    init = np.asarray(last_input_init, dtype=np.float32)
    filt = np.asarray(filt, dtype=np.float32)

    xc = np.concatenate(
        [np.broadcast_to(init[None], (B, CLEN, D)), x], axis=1
    )  # [B, 4608, D]
    xh = (xc * np.float32(1.0 / SCALE)).astype(np.float16)
    xr = xh.reshape(B, MIN, 128, D)                      # [b, m, j, ch]
    xt = xr.transpose(3, 2, 1, 0)                        # [ch, j, m, b]
    xt = xt.reshape(D, 128, XW)                          # [ch, j, m*4+b]

    fs = (filt * np.float32(SCALE)).astype(np.float16)   # [512, D]
    pf = np.zeros((D, 767), np.float16)
    pf[:, 127:639] = fs.T
    jj = np.arange(128)
    uu = np.arange(BW)
    idx = 127 - jj[:, None] + uu[None, :]                # [128, 640] in [0, 767)
    band = pf[:, idx]                                    # [ch, j, u]

    slab = np.empty((D, 128, CW), np.float16)
    slab[:, :, :XW] = xt
    slab[:, :, XW:] = band
    slab = slab.reshape(NCORES, NG, GCH, 128, CW).transpose(0, 1, 3, 2, 4)
    return np.ascontiguousarray(slab)


def kernel(x, last_input_init, filt):
    global LAST_RESULTS
    if "nc" not in _CACHE:
        _CACHE["nc"] = _build_bass()
    nc = _CACHE["nc"]

    slab = _prep_inputs(x, last_input_init, filt)
    in_maps = [{"slab": slab[core]} for core in range(NCORES)]

    trace = bool(os.environ.get("BASS_TRACE"))
    res = run_bass_kernel_spmd(nc, in_maps, list(range(NCORES)), trace=trace)
    LAST_RESULTS = res

    outs = []
    for core in range(NCORES):
        o = res.results[core]["out"].astype(np.float32)  # [NG, 128, GCH, 128]
        o = o.reshape(NG, 128, GCH, MOUT, B)             # [g, i, c, m0, b]
        o = o.transpose(4, 3, 1, 0, 2)                   # [b, m0, i, g, c]
        outs.append(o.reshape(B, L, DSH))
    out = np.concatenate(outs, axis=2)
    return np.ascontiguousarray(out, dtype=np.float32)


# revision 14
# speedup vs baseline: 1.2476x; 1.1272x over previous
"""Trainium2 Bass kernel for nn_CConv (causal depthwise FFT-conv, 512 taps).

The reference's FFT conv is exactly a causal depthwise conv1d with a
512-step learned init state prepended:
    out[b,t,c] = sum_k f[k,c] * xc[b, t+512-k, c],  xc = concat(init, x)

Mapping to the tensor engine (per channel c):
    out[i, (m0,b)] = sum_q  W_q[j,i].T @ X[j, (m0+q, b)]
where W_q[j,i] = f[i-j+128*(4-q), c] are 5 Toeplitz slices of a
[128 x 640] band built on the host, and X is the natural time-tiled
x (partition = t%128, columns = (time chunk, batch)).  The 5 matmuls
accumulate in PSUM.  Channels are sharded 8 ways across cores; inside a
core, channels stream in groups of 8 (fused x+band slab DMA -> 40
matmuls -> PSUM->SBUF copy -> output DMA).

Numerics: fp16 operands with fp32 PSUM accumulation.  The filter is
prescaled by 32 and x by 1/32 (exact powers of two; out = sum f*x
unchanged) so both operands sit far from the fp16 subnormal range.
"""

import os

import numpy as np

import concourse.bacc as bacc
import concourse.mybir as mybir
from concourse.bass_utils import run_bass_kernel_spmd
from concourse.tile import TileContext

B, L, D, CLEN = 4, 4096, 1024, 512
NCORES = 8
DSH = D // NCORES            # 128 channels per core
GCH = 8                      # channels per group
NG = DSH // GCH              # 16 groups per core
MIN = (CLEN + L) // 128      # 36 input chunks (4 init + 32 x)
MOUT = L // 128              # 32 output blocks
NQ = 5                       # contraction chunks per output block
XW = MIN * B                 # 144 x columns per channel
BW = 640                     # band columns per channel
CW = XW + BW                 # 784 slab columns per channel
GB = 4                       # groups per output DMA batch
SCALE = 32.0

_CACHE = {}
LAST_RESULTS = None          # BassKernelResults of the most recent run


def _build_bass():
    # Bacc (not plain Bass): its compile() legalizes sync waits (>1 wait per
    # instruction gets split into InstEventSemaphore), which walrus requires.
    nc = bacc.Bacc(None, target_bir_lowering=False)
    f16, f32 = mybir.dt.float16, mybir.dt.float32
    sd = nc.declare_dram_parameter("slab", [NG, 128, GCH, CW], f16, isOutput=False)
    od = nc.declare_dram_parameter(
        "out", [NG // GB, 128, GB, GCH, 128], f16, isOutput=True
    )

    with TileContext(nc) as tc:
        with (
            tc.tile_pool(name="sp", bufs=4) as sp,
            tc.tile_pool(name="op", bufs=4) as op,
            tc.tile_pool(name="pp", bufs=4, space="PSUM") as pp,
        ):
            for g in range(NG):
                st = sp.tile([128, GCH, CW], f16)
                if g == 0:
                    # split so the first channels' matmuls start sooner
                    nc.sync.dma_start(out=st[:, :2], in_=sd[g, :, :2])
                    nc.sync.dma_start(out=st[:, 2:], in_=sd[g, :, 2:])
                else:
                    nc.sync.dma_start(out=st[:], in_=sd[g])
                if g % GB == 0:
                    ot = op.tile([128, GB, GCH, 128], f16)
                for c in range(GCH):
                    ps = pp.tile([128, 128], f32)
                    for q in range(NQ):
                        nc.tensor.matmul(
                            ps[:],
                            lhsT=st[:, c, XW + 128 * (4 - q) : XW + 128 * (5 - q)],
                            rhs=st[:, c, 4 * q : 4 * q + 128],
                            start=(q == 0),
                            stop=(q == NQ - 1),
                        )
                    nc.vector.tensor_copy(out=ot[:, g % GB, c, :], in_=ps[:])
                if g % GB == GB - 1:
                    nc.sync.dma_start(out=od[g // GB], in_=ot[:])
    nc.finalize()  # Bacc.compile(): reg alloc + sync-wait legalization
    return nc


def _prep_inputs(x, last_input_init, filt):
    """Host-side: cast/scale to fp16 and prearrange into the exact SBUF
    layout so every DMA is a contiguous line-rate copy.

    slab[core, g, j, c, 0:144]   = xc[b, 128*m + j, ch] / 32   at col m*4+b
    slab[core, g, j, c, 144:784] = 32*f[u - j, ch]             at col 144+u
    """
    x = np.asarray(x, dtype=np.float32)
    init = np.asarray(last_input_init, dtype=np.float32)
    filt = np.asarray(filt, dtype=np.float32)

    xc = np.concatenate(
        [np.broadcast_to(init[None], (B, CLEN, D)), x], axis=1
    )  # [B, 4608, D]
    xh = (xc * np.float32(1.0 / SCALE)).astype(np.float16)
    xr = xh.reshape(B, MIN, 128, D)                      # [b, m, j, ch]
    xt = xr.transpose(3, 2, 1, 0)                        # [ch, j, m, b]
    xt = xt.reshape(D, 128, XW)                          # [ch, j, m*4+b]

    fs = (filt * np.float32(SCALE)).astype(np.float16)   # [512, D]
    pf = np.zeros((D, 767), np.float16)
    pf[:, 127:639] = fs.T
    jj = np.arange(128)
    uu = np.arange(BW)
    idx = 127 - jj[:, None] + uu[None, :]                # [128, 640] in [0, 767)
    band = pf[:, idx]                                    # [ch, j, u]

    slab = np.empty((D, 128, CW), np.float16)
    slab[:, :, :XW] = xt
    slab[:, :, XW:] = band
    slab = slab.reshape(NCORES, NG, GCH, 128, CW).transpose(0, 1, 3, 2, 4)
    return np.ascontiguousarray(slab)


def kernel(x, last_input_init, filt):
    global LAST_RESULTS
    if "nc" not in _CACHE:
        _CACHE["nc"] = _build_bass()
    nc = _CACHE["nc"]

    slab = _prep_inputs(x, last_input_init, filt)
    in_maps = [{"slab": slab[core]} for core in range(NCORES)]

    trace = bool(os.environ.get("BASS_TRACE"))
    res = run_bass_kernel_spmd(nc, in_maps, list(range(NCORES)), trace=trace)
    LAST_RESULTS = res

    outs = []
    for core in range(NCORES):
        o = res.results[core]["out"].astype(np.float32)  # [NG/GB, 128, GB, GCH, 128]
        o = o.transpose(0, 2, 1, 3, 4)                   # [NG/GB, GB, 128, GCH, 128]
        o = o.reshape(NG, 128, GCH, MOUT, B)             # [g, i, c, m0, b]
        o = o.transpose(4, 3, 1, 0, 2)                   # [b, m0, i, g, c]
        outs.append(o.reshape(B, L, DSH))
    out = np.concatenate(outs, axis=2)
    return np.ascontiguousarray(out, dtype=np.float32)


# revision 17
# speedup vs baseline: 1.2913x; 1.0350x over previous
"""Trainium2 Bass kernel for nn_CConv (causal depthwise FFT-conv, 512 taps).

The reference's FFT conv is exactly a causal depthwise conv1d with a
512-step learned init state prepended:
    out[b,t,c] = sum_k f[k,c] * xc[b, t+512-k, c],  xc = concat(init, x)

Mapping to the tensor engine (per channel c):
    out[i, (m0,b)] = sum_q  W_q[j,i].T @ X[j, (m0+q, b)]
where W_q[j,i] = f[i-j+128*(4-q), c] are 5 Toeplitz slices of a
[128 x 640] band built on the host, and X is the natural time-tiled
x (partition = t%128, columns = (time chunk, batch)).  The 5 matmuls
accumulate in PSUM.  Channels are sharded 8 ways across cores; inside a
core, channels stream in groups of 8 (fused x+band slab DMA -> 40
matmuls -> PSUM->SBUF copy -> output DMA).

Numerics: fp16 operands with fp32 PSUM accumulation.  The filter is
prescaled by 32 and x by 1/32 (exact powers of two; out = sum f*x
unchanged) so both operands sit far from the fp16 subnormal range.
"""

import os

import numpy as np

import concourse.bacc as bacc
import concourse.mybir as mybir
from concourse.bass_utils import run_bass_kernel_spmd
from concourse.tile import TileContext

B, L, D, CLEN = 4, 4096, 1024, 512
NCORES = 8
DSH = D // NCORES            # 128 channels per core
GCH = 8                      # channels per group
NG = DSH // GCH              # 16 groups per core
MIN = (CLEN + L) // 128      # 36 input chunks (4 init + 32 x)
MOUT = L // 128              # 32 output blocks
NQ = 5                       # contraction chunks per output block
XW = MIN * B                 # 144 x columns per channel
BW = 640                     # band columns per channel
CW = XW + BW                 # 784 slab columns per channel
GB = 4                       # groups per output DMA batch
SCALE = 32.0

_CACHE = {}
LAST_RESULTS = None          # BassKernelResults of the most recent run


def _build_bass():
    # Bacc (not plain Bass): its compile() legalizes sync waits (>1 wait per
    # instruction gets split into InstEventSemaphore), which walrus requires.
    nc = bacc.Bacc(None, target_bir_lowering=False)
    f16, f32 = mybir.dt.float16, mybir.dt.float32
    sd = nc.declare_dram_parameter("slab", [NG, 128, GCH, CW], f16, isOutput=False)
    # flat output, one region per group; late groups get their own small DMA
    # so the kernel tail isn't gated on a large final batch
    od = nc.declare_dram_parameter("out", [NG, 128, GCH, 128], f16, isOutput=True)

    with TileContext(nc) as tc:
        with (
            tc.tile_pool(name="sp", bufs=4) as sp,
            tc.tile_pool(name="op", bufs=4) as op,
            tc.tile_pool(name="pp", bufs=4, space="PSUM") as pp,
        ):
            batches = [(0, 4), (4, 4), (8, 4), (12, 2), (14, 1), (15, 1)]
            for g0, nb in batches:
                ot = op.tile([128, nb, GCH, 128], f16)
                for k in range(nb):
                    g = g0 + k
                    st = sp.tile([128, GCH, CW], f16)
                    if g == 0:
                        # split so the first channels' matmuls start sooner
                        nc.sync.dma_start(out=st[:, :2], in_=sd[g, :, :2])
                        nc.sync.dma_start(out=st[:, 2:], in_=sd[g, :, 2:])
                    else:
                        nc.sync.dma_start(out=st[:], in_=sd[g])
                    for c in range(GCH):
                        ps = pp.tile([128, 128], f32)
                        for q in range(NQ):
                            nc.tensor.matmul(
                                ps[:],
                                lhsT=st[:, c, XW + 128 * (4 - q) : XW + 128 * (5 - q)],
                                rhs=st[:, c, 4 * q : 4 * q + 128],
                                start=(q == 0),
                                stop=(q == NQ - 1),
                            )
                        nc.vector.tensor_copy(out=ot[:, k, c, :], in_=ps[:])
                nc.sync.dma_start(
                    out=od[g0 : g0 + nb].transpose([1, 0, 2, 3]), in_=ot[:]
                )
    nc.finalize()  # Bacc.compile(): reg alloc + sync-wait legalization
    return nc


def _prep_inputs(x, last_input_init, filt):
    """Host-side: cast/scale to fp16 and prearrange into the exact SBUF
    layout so every DMA is a contiguous line-rate copy.

    slab[core, g, j, c, 0:144]   = xc[b, 128*m + j, ch] / 32   at col m*4+b
    slab[core, g, j, c, 144:784] = 32*f[u - j, ch]             at col 144+u
    """
    x = np.asarray(x, dtype=np.float32)
    init = np.asarray(last_input_init, dtype=np.float32)
    filt = np.asarray(filt, dtype=np.float32)

    xc = np.concatenate(
        [np.broadcast_to(init[None], (B, CLEN, D)), x], axis=1
    )  # [B, 4608, D]
    xh = (xc * np.float32(1.0 / SCALE)).astype(np.float16)
    xr = xh.reshape(B, MIN, 128, D)                      # [b, m, j, ch]
    xt = xr.transpose(3, 2, 1, 0)                        # [ch, j, m, b]
    xt = xt.reshape(D, 128, XW)                          # [ch, j, m*4+b]

    fs = (filt * np.float32(SCALE)).astype(np.float16)   # [512, D]
    pf = np.zeros((D, 767), np.float16)
    pf[:, 127:639] = fs.T
    jj = np.arange(128)
    uu = np.arange(BW)
    idx = 127 - jj[:, None] + uu[None, :]                # [128, 640] in [0, 767)
    band = pf[:, idx]                                    # [ch, j, u]

    slab = np.empty((D, 128, CW), np.float16)
    slab[:, :, :XW] = xt
    slab[:, :, XW:] = band
    slab = slab.reshape(NCORES, NG, GCH, 128, CW).transpose(0, 1, 3, 2, 4)
    return np.ascontiguousarray(slab)


def kernel(x, last_input_init, filt):
    global LAST_RESULTS
    if "nc" not in _CACHE:
        _CACHE["nc"] = _build_bass()
    nc = _CACHE["nc"]

    slab = _prep_inputs(x, last_input_init, filt)
    in_maps = [{"slab": slab[core]} for core in range(NCORES)]

    trace = bool(os.environ.get("BASS_TRACE"))
    res = run_bass_kernel_spmd(nc, in_maps, list(range(NCORES)), trace=trace)
    LAST_RESULTS = res

    outs = []
    for core in range(NCORES):
        o = res.results[core]["out"].astype(np.float32)  # [NG, 128, GCH, 128]
        o = o.reshape(NG, 128, GCH, MOUT, B)             # [g, i, c, m0, b]
        o = o.transpose(4, 3, 1, 0, 2)                   # [b, m0, i, g, c]
        outs.append(o.reshape(B, L, DSH))
    out = np.concatenate(outs, axis=2)
    return np.ascontiguousarray(out, dtype=np.float32)
